# revision 16
# baseline (speedup 1.0000x reference)
"""Trainium2 Bass kernel for nn_AssociatorLoss.

Reference computation (B=32, N=32), a = cayley_cube (B,N,N,N), rows a[b,i,j,:]
are probability distributions:

    one[b,i,j,k,l] = sum_m a[b,i,m,l] * a[b,j,k,m]
    two[b,i,j,k,l] = sum_m a[b,m,k,l] * a[b,i,j,m]
    kl = sum(two * (log(two) - log(one))) / B

Strategy (data-parallel over b, 4 batch elements per core, no collectives —
the 8 per-core partial sums are combined on the host):

Per batch element, with x,y,z the three trailing axes of a[b]:
  A  = a[b] in SBUF as [x, (y,z)]        (natural, 32 partitions x 1024)
  AT = 32x32-block transpose of A  -> [z, (y,x)]
  AY = block transpose of A viewed with free dims swapped -> [y, (z,x)]

Matmuls (K = m = 32, bf16, PE):
  two  chunk c (i in [4c,4c+4)):  out[p=(i,j), f=(k,l)] :
       lhsT[m,(i,j)] = AT viewed [z,x,y][:, 4c:4c+4, :],  rhs[m,(k,l)] = A
  one  chunk c:                   out[p=(i,l), f=(k,j)] :
       lhsT[m,(i,l)] = AY viewed [y,x,z][:, 4c:4c+4, :],  rhs[m,(k,j)] = AT

  ("one" comes out with free index (k,j) so that the 32x32-block transpose of
   the "two" chunk — which maps [p=(i,j),f=(k,l)] -> [p=(i,l),f=(k,j)] —
   aligns elementwise with it.)

Elementwise/reduction per chunk:
  ACT:    LT = Ln(two_psum) -> bf16, LO = Ln(one_psum) -> bf16
  DVE:    twot = block-transpose(two_psum);  ttr: P = two_psum*LT, rowsum -> accP
  GPSIMD: stt:  P2 = twot*LO, rowsum -> accM
KL = (sum(accP) - sum(accM)) / B, finished on host in float64.
"""

import sys

for _p in ("/opt/trn_rl_repo",):
    if _p not in sys.path:
        sys.path.insert(0, _p)

import numpy as np

import concourse.bacc as bacc
import concourse.mybir as mybir
import concourse.tile as tile
from concourse.bass_utils import run_bass_kernel_spmd

B, N = 32, 32
N_CORES = 8
B_LOCAL = B // N_CORES  # 4
NCHUNK = (N * N) // 128  # 8 chunks of 128 rows per batch element
F32 = mybir.dt.float32
BF16 = mybir.dt.bfloat16
POOL_COLS = 768


def build(b_local=B_LOCAL, mm_dtype=BF16, log_dtype=F32, reps=1, sub_pool=896, skip=(), loop_reps=0):
    nc = bacc.Bacc(None, target_bir_lowering=False)
    ncols = b_local * NCHUNK
    a_ext = nc.declare_dram_parameter("cayley_cube", [b_local, N, N, N], F32, isOutput=False)
    out_ext = nc.declare_dram_parameter("out", [128, ncols], F32, isOutput=True)
    av = a_ext.rearrange("b x y z -> b x (y z)")

    mult = mybir.AluOpType.mult
    add = mybir.AluOpType.add
    subtract = mybir.AluOpType.subtract

    with tile.TileContext(nc) as tc:
        with (
            tc.tile_pool(name="apool", bufs=2) as apool,
            tc.tile_pool(name="spool", bufs=3) as spool,
            tc.tile_pool(name="scratch", bufs=1) as scratch,
            tc.tile_pool(name="acc", bufs=1) as accpool,
            tc.tile_pool(name="psumT", bufs=2, space="PSUM") as psumT,
        ):
            accP = accpool.tile([128, ncols], F32)
            p1 = scratch.tile([128, 1024], BF16)

            import contextlib
            loop_ctx = tc.For_i(0, loop_reps, 1) if loop_reps else contextlib.nullcontext()
            with loop_ctx:
             for _rep in range(reps):
              for b in range(b_local):
                a32 = apool.tile([N, 1024], F32, tag="a32")
                nc.sync.dma_start(out=a32[:], in_=av[b])
                ab = apool.tile([N, 1024], mm_dtype, tag="ab")
                nc.vector.tensor_copy(ab[:], a32[:])
                # at[z, y*32+x] = a[x,y,z]  (O-matmul rhs: n = k*32+j)
                at = apool.tile([N, 1024], mm_dtype, tag="at")
                nc.vector.transpose(at[:], ab[:])
                # at2[z, x*32+y] = a[x,y,z]  (T-matmul stationary operand:
                # contiguous 128-col slices enumerate (i-group, j))
                at2 = apool.tile([N, 1024], mm_dtype, tag="at2")
                nc.gpsimd.tensor_copy(
                    at2[:].rearrange("p (x y) -> p y x", x=N, y=N),
                    at[:].rearrange("p (y x) -> p y x", y=N, x=N),
                )
                # ay2[y, x*32+z] = a[x,y,z]  (O-matmul stationary operand)
                ay2 = apool.tile([N, 1024], mm_dtype, tag="ay2")
                nc.vector.transpose(ay2[:], at2[:])

                for c in range(NCHUNK):
                    col = b * NCHUNK + c
                    # merged PSUM tile: two in [:, 0:1024], one in [:, 1024:2048]
                    tpo = psumT.tile([128, 2048], F32, tag="tpo")
                    tp = tpo[:, 0:1024]
                    op = tpo[:, 1024:2048]
                    ms = slice(128 * c, 128 * (c + 1))
                    for h in range(2):
                        cs = slice(512 * h, 512 * (h + 1))
                        nc.tensor.matmul(tp[:, cs], at2[:, ms], ab[:, cs],
                                         start=True, stop=True)
                        nc.tensor.matmul(op[:, cs], ay2[:, ms], at[:, cs],
                                         start=True, stop=True)

                    # one Ln pass over both: LTO = [ln(two) | ln(one)] bf16
                    lto = spool.tile([128, 2048], BF16, tag="lto")
                    nc.scalar.activation(lto[:], tpo[:], mybir.ActivationFunctionType.Ln)

                    # align ln(one) with two's layout via 32x32-block transpose
                    lot = spool.tile([128, 1024], BF16, tag="lot")
                    nc.vector.transpose(lot[:], lto[:, 1024:2048])

                    # D = ln(two) - ln(one)_aligned  (bf16; Pool takes first
                    # sub_pool cols, DVE the rest)
                    dd = spool.tile([128, 1024], BF16, tag="dd")
                    if sub_pool > 0:
                        nc.gpsimd.tensor_tensor(
                            out=dd[:, 0:sub_pool], in0=lto[:, 0:sub_pool],
                            in1=lot[:, 0:sub_pool], op=subtract,
                        )
                    if sub_pool < 1024:
                        nc.vector.tensor_tensor(
                            out=dd[:, sub_pool:1024], in0=lto[:, sub_pool:1024],
                            in1=lot[:, sub_pool:1024], op=subtract,
                        )

                    # single fused dot: sum two * D -> accP column
                    nc.vector.scalar_tensor_tensor(
                        out=p1[:], in0=tp, scalar=1.0, in1=dd[:],
                        op0=mult, op1=mult, accum_out=accP[:, col:col + 1],
                    )

            nc.sync.dma_start(out=out_ext[:, 0:ncols], in_=accP[:])

    nc.compile()
    return nc


def kernel(cayley_cube: np.ndarray) -> np.ndarray:
    assert cayley_cube.shape == (B, N, N, N)
    nc = build()
    shards = cayley_cube.reshape(N_CORES, B_LOCAL, N, N, N)
    in_maps = [
        {"cayley_cube": np.ascontiguousarray(shards[i])} for i in range(N_CORES)
    ]
    res = run_bass_kernel_spmd(nc, in_maps, core_ids=list(range(N_CORES)))
    ncols = B_LOCAL * NCHUNK
    tot = np.float64(0.0)
    for r in res.results:
        acc = r["out"]
        tot += acc[:, :ncols].sum(dtype=np.float64)
    return np.float32(tot / B)


if __name__ == "__main__":
    rng = np.random.default_rng(0)
    raw = rng.uniform(0.05, 1.0, size=(B, N, N, N)).astype(np.float32)
    a = raw / raw.sum(axis=-1, keepdims=True)
    print(kernel(a))


# revision 17
# speedup vs baseline: 1.0437x; 1.0437x over previous
"""Trainium2 Bass kernel for nn_AssociatorLoss.

Reference computation (B=32, N=32), a = cayley_cube (B,N,N,N), rows a[b,i,j,:]
are probability distributions:

    one[b,i,j,k,l] = sum_m a[b,i,m,l] * a[b,j,k,m]
    two[b,i,j,k,l] = sum_m a[b,m,k,l] * a[b,i,j,m]
    kl = sum(two * (log(two) - log(one))) / B

Strategy (data-parallel over b, 4 batch elements per core, no collectives —
the 8 per-core partial sums are combined on the host):

Per batch element, with x,y,z the three trailing axes of a[b]:
  A  = a[b] in SBUF as [x, (y,z)]        (natural, 32 partitions x 1024)
  AT = 32x32-block transpose of A  -> [z, (y,x)]
  AY = block transpose of A viewed with free dims swapped -> [y, (z,x)]

Matmuls (K = m = 32, bf16, PE):
  two  chunk c (i in [4c,4c+4)):  out[p=(i,j), f=(k,l)] :
       lhsT[m,(i,j)] = AT viewed [z,x,y][:, 4c:4c+4, :],  rhs[m,(k,l)] = A
  one  chunk c:                   out[p=(i,l), f=(k,j)] :
       lhsT[m,(i,l)] = AY viewed [y,x,z][:, 4c:4c+4, :],  rhs[m,(k,j)] = AT

  ("one" comes out with free index (k,j) so that the 32x32-block transpose of
   the "two" chunk — which maps [p=(i,j),f=(k,l)] -> [p=(i,l),f=(k,j)] —
   aligns elementwise with it.)

Elementwise/reduction per chunk:
  ACT:    LT = Ln(two_psum) -> bf16, LO = Ln(one_psum) -> bf16
  DVE:    twot = block-transpose(two_psum);  ttr: P = two_psum*LT, rowsum -> accP
  GPSIMD: stt:  P2 = twot*LO, rowsum -> accM
KL = (sum(accP) - sum(accM)) / B, finished on host in float64.
"""

import sys

for _p in ("/opt/trn_rl_repo",):
    if _p not in sys.path:
        sys.path.insert(0, _p)

import numpy as np

import concourse.bacc as bacc
import concourse.mybir as mybir
import concourse.tile as tile
from concourse.bass_utils import run_bass_kernel_spmd

B, N = 32, 32
N_CORES = 8
B_LOCAL = B // N_CORES  # 4
NCHUNK = (N * N) // 128  # 8 chunks of 128 rows per batch element
F32 = mybir.dt.float32
BF16 = mybir.dt.bfloat16
POOL_COLS = 768


def build(b_local=B_LOCAL, mm_dtype=BF16, log_dtype=F32, reps=1, sub_pool=0, skip=(), loop_reps=0):
    nc = bacc.Bacc(None, target_bir_lowering=False)
    ncols = b_local * NCHUNK
    a_ext = nc.declare_dram_parameter("cayley_cube", [b_local, N, N, N], F32, isOutput=False)
    out_ext = nc.declare_dram_parameter("out", [128, ncols], F32, isOutput=True)
    av = a_ext.rearrange("b x y z -> b x (y z)")

    mult = mybir.AluOpType.mult
    add = mybir.AluOpType.add
    subtract = mybir.AluOpType.subtract

    with tile.TileContext(nc) as tc:
        with (
            tc.tile_pool(name="apool", bufs=2) as apool,
            tc.tile_pool(name="spool", bufs=4) as spool,
            tc.tile_pool(name="scratch", bufs=1) as scratch,
            tc.tile_pool(name="acc", bufs=1) as accpool,
            tc.tile_pool(name="psumT", bufs=2, space="PSUM") as psumT,
            tc.tile_pool(name="psumO", bufs=2, space="PSUM") as psumO,
        ):
            accP = accpool.tile([128, ncols], F32)
            p1 = scratch.tile([128, 1024], BF16)

            import contextlib
            loop_ctx = tc.For_i(0, loop_reps, 1) if loop_reps else contextlib.nullcontext()
            with loop_ctx:
             for _rep in range(reps):
              for b in range(b_local):
                a32 = apool.tile([N, 1024], F32, tag="a32")
                nc.sync.dma_start(out=a32[:], in_=av[b])
                ab = apool.tile([N, 1024], mm_dtype, tag="ab")
                nc.vector.tensor_copy(ab[:], a32[:])
                # at[z, y*32+x] = a[x,y,z]  (O-matmul rhs: n = k*32+j)
                at = apool.tile([N, 1024], mm_dtype, tag="at")
                nc.vector.transpose(at[:], ab[:])
                # at2[z, x*32+y] = a[x,y,z]  (T-matmul stationary operand:
                # contiguous 128-col slices enumerate (i-group, j))
                at2 = apool.tile([N, 1024], mm_dtype, tag="at2")
                nc.gpsimd.tensor_copy(
                    at2[:].rearrange("p (x y) -> p y x", x=N, y=N),
                    at[:].rearrange("p (y x) -> p y x", y=N, x=N),
                )
                # ay2[y, x*32+z] = a[x,y,z]  (O-matmul stationary operand)
                ay2 = apool.tile([N, 1024], mm_dtype, tag="ay2")
                nc.vector.transpose(ay2[:], at2[:])

                for c in range(NCHUNK):
                    col = b * NCHUNK + c
                    tp = psumT.tile([128, 1024], F32, tag="tp")
                    op = psumO.tile([128, 1024], F32, tag="op")
                    ms = slice(128 * c, 128 * (c + 1))
                    for h in range(2):
                        cs = slice(512 * h, 512 * (h + 1))
                        nc.tensor.matmul(tp[:, cs], at2[:, ms], ab[:, cs],
                                         start=True, stop=True)
                        nc.tensor.matmul(op[:, cs], ay2[:, ms], at[:, cs],
                                         start=True, stop=True)

                    # ACT stages everything out of PSUM immediately (bf16):
                    # tb = two, lt = ln(two), lo = ln(one)
                    tb = spool.tile([128, 1024], BF16, tag="tb")
                    nc.scalar.copy(tb[:], tp[:])
                    lt = spool.tile([128, 1024], BF16, tag="lt")
                    nc.scalar.activation(lt[:], tp[:], mybir.ActivationFunctionType.Ln)
                    lo = spool.tile([128, 1024], BF16, tag="lo")
                    nc.scalar.activation(lo[:], op[:], mybir.ActivationFunctionType.Ln)

                    # align ln(one) with two's layout via 32x32-block transpose
                    lot = spool.tile([128, 1024], BF16, tag="lot")
                    nc.vector.transpose(lot[:], lo[:])

                    # D = ln(two) - ln(one)_aligned (bf16; optional Pool slice)
                    dd = spool.tile([128, 1024], BF16, tag="dd")
                    if sub_pool > 0:
                        nc.gpsimd.tensor_tensor(
                            out=dd[:, 0:sub_pool], in0=lt[:, 0:sub_pool],
                            in1=lot[:, 0:sub_pool], op=subtract,
                        )
                    if sub_pool < 1024:
                        nc.vector.tensor_tensor(
                            out=dd[:, sub_pool:1024], in0=lt[:, sub_pool:1024],
                            in1=lot[:, sub_pool:1024], op=subtract,
                        )

                    # single fused dot: sum two * D -> accP column
                    nc.vector.scalar_tensor_tensor(
                        out=p1[:], in0=tb[:], scalar=1.0, in1=dd[:],
                        op0=mult, op1=mult, accum_out=accP[:, col:col + 1],
                    )

            nc.sync.dma_start(out=out_ext[:, 0:ncols], in_=accP[:])

    nc.compile()
    return nc


def kernel(cayley_cube: np.ndarray) -> np.ndarray:
    assert cayley_cube.shape == (B, N, N, N)
    nc = build()
    shards = cayley_cube.reshape(N_CORES, B_LOCAL, N, N, N)
    in_maps = [
        {"cayley_cube": np.ascontiguousarray(shards[i])} for i in range(N_CORES)
    ]
    res = run_bass_kernel_spmd(nc, in_maps, core_ids=list(range(N_CORES)))
    ncols = B_LOCAL * NCHUNK
    tot = np.float64(0.0)
    for r in res.results:
        acc = r["out"]
        tot += acc[:, :ncols].sum(dtype=np.float64)
    return np.float32(tot / B)


if __name__ == "__main__":
    rng = np.random.default_rng(0)
    raw = rng.uniform(0.05, 1.0, size=(B, N, N, N)).astype(np.float32)
    a = raw / raw.sum(axis=-1, keepdims=True)
    print(kernel(a))


# revision 18
# speedup vs baseline: 1.6865x; 1.6159x over previous
"""Trainium2 Bass kernel for nn_AssociatorLoss.

Reference computation (B=32, N=32), a = cayley_cube (B,N,N,N), rows a[b,i,j,:]
are probability distributions:

    one[b,i,j,k,l] = sum_m a[b,i,m,l] * a[b,j,k,m]
    two[b,i,j,k,l] = sum_m a[b,m,k,l] * a[b,i,j,m]
    kl = sum(two * (log(two) - log(one))) / B

Strategy (data-parallel over b, 4 batch elements per core, no collectives —
the 8 per-core partial sums are combined on the host):

Per batch element, with x,y,z the three trailing axes of a[b]:
  A  = a[b] in SBUF as [x, (y,z)]        (natural, 32 partitions x 1024)
  AT = 32x32-block transpose of A  -> [z, (y,x)]
  AY = block transpose of A viewed with free dims swapped -> [y, (z,x)]

Matmuls (K = m = 32, bf16, PE):
  two  chunk c (i in [4c,4c+4)):  out[p=(i,j), f=(k,l)] :
       lhsT[m,(i,j)] = AT viewed [z,x,y][:, 4c:4c+4, :],  rhs[m,(k,l)] = A
  one  chunk c:                   out[p=(i,l), f=(k,j)] :
       lhsT[m,(i,l)] = AY viewed [y,x,z][:, 4c:4c+4, :],  rhs[m,(k,j)] = AT

  ("one" comes out with free index (k,j) so that the 32x32-block transpose of
   the "two" chunk — which maps [p=(i,j),f=(k,l)] -> [p=(i,l),f=(k,j)] —
   aligns elementwise with it.)

Elementwise/reduction per chunk:
  ACT:    LT = Ln(two_psum) -> bf16, LO = Ln(one_psum) -> bf16
  DVE:    twot = block-transpose(two_psum);  ttr: P = two_psum*LT, rowsum -> accP
  GPSIMD: stt:  P2 = twot*LO, rowsum -> accM
KL = (sum(accP) - sum(accM)) / B, finished on host in float64.
"""

import sys

for _p in ("/opt/trn_rl_repo",):
    if _p not in sys.path:
        sys.path.insert(0, _p)

import numpy as np

import concourse.bacc as bacc
import concourse.mybir as mybir
import concourse.tile as tile
from concourse.bass_utils import run_bass_kernel_spmd

B, N = 32, 32
N_CORES = 8
B_LOCAL = B // N_CORES  # 4
NCHUNK = (N * N) // 128  # 8 chunks of 128 rows per batch element
F32 = mybir.dt.float32
BF16 = mybir.dt.bfloat16
POOL_COLS = 768


def build(b_local=B_LOCAL, mm_dtype=BF16, log_dtype=F32, reps=1, sub_pool=0, skip=(), loop_reps=0):
    nc = bacc.Bacc(None, target_bir_lowering=False)
    ncols = b_local * NCHUNK
    a_ext = nc.declare_dram_parameter("cayley_cube", [b_local, N, N, N], F32, isOutput=False)
    out_ext = nc.declare_dram_parameter("out", [128, ncols], F32, isOutput=True)
    av = a_ext.rearrange("b x y z -> b x (y z)")

    mult = mybir.AluOpType.mult
    add = mybir.AluOpType.add
    subtract = mybir.AluOpType.subtract

    with tile.TileContext(nc) as tc:
        with (
            tc.tile_pool(name="apool", bufs=2) as apool,
            tc.tile_pool(name="spool", bufs=8) as spool,
            tc.tile_pool(name="scratch", bufs=1) as scratch,
            tc.tile_pool(name="acc", bufs=1) as accpool,
            tc.tile_pool(name="psumT", bufs=2, space="PSUM") as psumT,
            tc.tile_pool(name="psumO", bufs=2, space="PSUM") as psumO,
        ):
            accP = accpool.tile([128, ncols], F32)
            p1 = scratch.tile([128, 1024], BF16)

            import contextlib
            loop_ctx = tc.For_i(0, loop_reps, 1) if loop_reps else contextlib.nullcontext()
            with loop_ctx:
             for _rep in range(reps):
              for b in range(b_local):
                a32 = apool.tile([N, 1024], F32, tag="a32")
                nc.sync.dma_start(out=a32[:], in_=av[b])
                ab = apool.tile([N, 1024], mm_dtype, tag="ab")
                nc.vector.tensor_copy(ab[:], a32[:])
                # at[z, y*32+x] = a[x,y,z]  (O-matmul rhs: n = k*32+j)
                at = apool.tile([N, 1024], mm_dtype, tag="at")
                nc.vector.transpose(at[:], ab[:])
                # at2[z, x*32+y] = a[x,y,z]  (T-matmul stationary operand:
                # contiguous 128-col slices enumerate (i-group, j))
                at2 = apool.tile([N, 1024], mm_dtype, tag="at2")
                nc.gpsimd.tensor_copy(
                    at2[:].rearrange("p (x y) -> p y x", x=N, y=N),
                    at[:].rearrange("p (y x) -> p y x", y=N, x=N),
                )
                # ay2[y, x*32+z] = a[x,y,z]  (O-matmul stationary operand)
                ay2 = apool.tile([N, 1024], mm_dtype, tag="ay2")
                nc.vector.transpose(ay2[:], at2[:])

                for c in range(NCHUNK):
                    col = b * NCHUNK + c
                    tp = psumT.tile([128, 1024], F32, tag="tp")
                    op = psumO.tile([128, 1024], F32, tag="op")
                    ms = slice(128 * c, 128 * (c + 1))
                    for h in range(2):
                        cs = slice(512 * h, 512 * (h + 1))
                        nc.tensor.matmul(tp[:, cs], at2[:, ms], ab[:, cs],
                                         start=True, stop=True)
                        nc.tensor.matmul(op[:, cs], ay2[:, ms], at[:, cs],
                                         start=True, stop=True)

                    # ACT stages everything out of PSUM immediately (bf16):
                    # tb = two, lt = ln(two), lo = ln(one)
                    lo = spool.tile([128, 1024], BF16, tag="lo")
                    nc.scalar.activation(lo[:], op[:], mybir.ActivationFunctionType.Ln)
                    lt = spool.tile([128, 1024], BF16, tag="lt")
                    nc.scalar.activation(lt[:], tp[:], mybir.ActivationFunctionType.Ln)
                    tb = spool.tile([128, 1024], BF16, tag="tb")
                    nc.scalar.copy(tb[:], tp[:])

                    # align ln(one) with two's layout via 32x32-block transpose
                    lot = spool.tile([128, 1024], BF16, tag="lot")
                    nc.vector.transpose(lot[:], lo[:])

                    # D = ln(two) - ln(one)_aligned (bf16; optional Pool slice)
                    dd = spool.tile([128, 1024], BF16, tag="dd")
                    if sub_pool > 0:
                        nc.gpsimd.tensor_tensor(
                            out=dd[:, 0:sub_pool], in0=lt[:, 0:sub_pool],
                            in1=lot[:, 0:sub_pool], op=subtract,
                        )
                    if sub_pool < 1024:
                        nc.vector.tensor_tensor(
                            out=dd[:, sub_pool:1024], in0=lt[:, sub_pool:1024],
                            in1=lot[:, sub_pool:1024], op=subtract,
                        )

                    # single fused dot: sum two * D -> accP column
                    nc.vector.scalar_tensor_tensor(
                        out=p1[:], in0=tb[:], scalar=1.0, in1=dd[:],
                        op0=mult, op1=mult, accum_out=accP[:, col:col + 1],
                    )

            nc.sync.dma_start(out=out_ext[:, 0:ncols], in_=accP[:])

    nc.compile()
    return nc


def kernel(cayley_cube: np.ndarray) -> np.ndarray:
    assert cayley_cube.shape == (B, N, N, N)
    nc = build()
    shards = cayley_cube.reshape(N_CORES, B_LOCAL, N, N, N)
    in_maps = [
        {"cayley_cube": np.ascontiguousarray(shards[i])} for i in range(N_CORES)
    ]
    res = run_bass_kernel_spmd(nc, in_maps, core_ids=list(range(N_CORES)))
    ncols = B_LOCAL * NCHUNK
    tot = np.float64(0.0)
    for r in res.results:
        acc = r["out"]
        tot += acc[:, :ncols].sum(dtype=np.float64)
    return np.float32(tot / B)


if __name__ == "__main__":
    rng = np.random.default_rng(0)
    raw = rng.uniform(0.05, 1.0, size=(B, N, N, N)).astype(np.float32)
    a = raw / raw.sum(axis=-1, keepdims=True)
    print(kernel(a))
